# revision 4
# baseline (speedup 1.0000x reference)
"""Trainium2 Bass kernel for channelwise EMA (exponential moving average).

Reference computation (per batch b, channel c):
    a = sigmoid(raw)          # [C]
    y[b, 0, c] = x[b, 0, c]
    y[b, t, c] = a[c] * x[b, t, c] + (1 - a[c]) * y[b, t-1, c]

Strategy
--------
* Shard batch B=16 across 8 cores (2 batches per core); replicate the
  tiny per-channel coefficient vectors.
* Host-side, pre-transpose each core's shard to channel-major
  [bpc, C, T] so time is contiguous — every DMA is then a fully
  contiguous 16KB-per-partition transfer (f32 DMA-transpose does not
  exist on TRN2, and AP-rearrangement DMAs are ~19x slower).
* On device, substitute z = y / a so the recurrence becomes
      z_t = d * z_{t-1} + x_t,      d = 1 - a
  which is exactly one native `tensor_tensor_scan` (DVE) per
  [128 channels, T] tile — no pre-scale pass over the input.
* The y_0 = x_0 boundary is absorbed into the scan's initial value:
  initial = x_0 / a  gives  z_0 = d*x_0/a + x_0 = x_0/a  =>  y_0 = x_0.
* Post-scale y = a * z runs on the Scalar (ACT) engine with a
  per-partition scale, in parallel with the DVE scans.
"""

import numpy as np

F32 = None  # set lazily (concourse import is heavy; keep module import light)


def _build_program(bpc: int, C: int, T: int, use_broadcast_ap: bool):
    import concourse.bacc as bacc
    import concourse.mybir as mybir
    from concourse.tile import TileContext

    f32 = mybir.dt.float32
    P = 128
    n_ctiles = C // P

    # Bacc (not raw Bass): its finalize() runs generate_event_semaphores,
    # which splits multi-sem waits — TRN2 allows at most 1 wait/instruction.
    nc = bacc.Bacc()
    xt = nc.declare_dram_parameter("xt", [bpc, C, T], f32, isOutput=False)
    a_pd = nc.declare_dram_parameter("a_pd", [P, n_ctiles], f32, isOutput=False)
    d_pd = nc.declare_dram_parameter("d_pd", [P, n_ctiles], f32, isOutput=False)
    ia_pd = nc.declare_dram_parameter("ia_pd", [P, n_ctiles], f32, isOutput=False)
    yt = nc.declare_dram_parameter("yt", [bpc, C, T], f32, isOutput=True)

    with TileContext(nc) as tc:
        with (
            tc.tile_pool(name="coef", bufs=1) as coef_pool,
            tc.tile_pool(name="dbc", bufs=2) as dpool,
            tc.tile_pool(name="xp", bufs=3) as xpool,
            tc.tile_pool(name="zp", bufs=2) as zpool,
            tc.tile_pool(name="yp", bufs=3) as ypool,
            tc.tile_pool(name="init", bufs=4) as spool,
        ):
            a_t = coef_pool.tile([P, n_ctiles], f32, tag="a")
            d_t = coef_pool.tile([P, n_ctiles], f32, tag="d")
            ia_t = coef_pool.tile([P, n_ctiles], f32, tag="ia")
            nc.sync.dma_start(out=a_t[:], in_=a_pd[:, :])
            nc.sync.dma_start(out=d_t[:], in_=d_pd[:, :])
            nc.sync.dma_start(out=ia_t[:], in_=ia_pd[:, :])

            if not use_broadcast_ap:
                ones = coef_pool.tile([P, T], f32, tag="ones")
                nc.vector.memset(ones[:], 1.0)

            for j in range(n_ctiles):
                cs = slice(j * P, (j + 1) * P)
                if use_broadcast_ap:
                    dbc_ap = d_t[:, j : j + 1].broadcast_to([P, T])
                else:
                    dbc = dpool.tile([P, T], f32)
                    # broadcast d[c] along the free dim on the ACT engine
                    nc.scalar.mul(dbc[:], ones[:], d_t[:, j : j + 1])
                    dbc_ap = dbc[:]
                for b in range(bpc):
                    x_tile = xpool.tile([P, T], f32)
                    nc.sync.dma_start(out=x_tile[:], in_=xt[b, cs, :])
                    init = spool.tile([P, 1], f32)
                    nc.vector.tensor_mul(
                        out=init[:], in0=x_tile[:, 0:1], in1=ia_t[:, j : j + 1]
                    )
                    z_tile = zpool.tile([P, T], f32)
                    nc.vector.tensor_tensor_scan(
                        out=z_tile[:],
                        data0=dbc_ap,
                        data1=x_tile[:],
                        initial=init[:],
                        op0=mybir.AluOpType.mult,
                        op1=mybir.AluOpType.add,
                    )
                    y_tile = ypool.tile([P, T], f32)
                    nc.scalar.mul(y_tile[:], z_tile[:], a_t[:, j : j + 1])
                    nc.sync.dma_start(out=yt[b, cs, :], in_=y_tile[:])
    nc.finalize()
    return nc


def _host_coeffs(raw: np.ndarray, P: int = 128):
    """sigmoid / complements in f64, packed [P, n_ctiles] with [p, j] = v[j*P + p]."""
    a64 = 1.0 / (1.0 + np.exp(-raw.astype(np.float64)))
    d64 = 1.0 - a64
    ia64 = 1.0 / a64
    C = raw.shape[0]
    n_ctiles = C // P

    def pack(v64):
        return np.ascontiguousarray(
            v64.astype(np.float32).reshape(n_ctiles, P).T
        )

    return pack(a64), pack(d64), pack(ia64)


USE_BROADCAST_AP = False


def kernel_with_results(
    x: np.ndarray,
    raw: np.ndarray,
    use_broadcast_ap: bool = USE_BROADCAST_AP,
    **run_kwargs,
):
    from concourse.bass_utils import run_bass_kernel_spmd

    B, T, C = x.shape
    n_cores = 8
    bpc = B // n_cores

    a_pd, d_pd, ia_pd = _host_coeffs(raw)

    nc = _build_program(bpc, C, T, use_broadcast_ap=use_broadcast_ap)

    in_maps = []
    for i in range(n_cores):
        shard = np.ascontiguousarray(
            x[i * bpc : (i + 1) * bpc].transpose(0, 2, 1)
        )  # [bpc, C, T], time contiguous
        in_maps.append({"xt": shard, "a_pd": a_pd, "d_pd": d_pd, "ia_pd": ia_pd})

    res = run_bass_kernel_spmd(nc, in_maps, core_ids=list(range(n_cores)), **run_kwargs)

    y = np.empty_like(x)
    for i in range(n_cores):
        y[i * bpc : (i + 1) * bpc] = res.results[i]["yt"].transpose(0, 2, 1)
    return y, res


def kernel(x: np.ndarray, raw: np.ndarray) -> np.ndarray:
    y, _ = kernel_with_results(x, raw)
    return y


# revision 6
# speedup vs baseline: 2.7249x; 2.7249x over previous
"""Trainium2 Bass kernel for channelwise EMA (exponential moving average).

Reference computation (per batch b, channel c):
    a = sigmoid(raw)          # [C]
    y[b, 0, c] = x[b, 0, c]
    y[b, t, c] = a[c] * x[b, t, c] + (1 - a[c]) * y[b, t-1, c]

Strategy
--------
* Shard batch B=16 across 8 cores (2 batches per core); replicate the
  tiny per-channel coefficient vectors.
* Host-side, pre-transpose each core's shard to channel-major
  [bpc, C, T] so time is contiguous — every DMA is then a fully
  contiguous 16KB-per-partition transfer (f32 DMA-transpose does not
  exist on TRN2, and AP-rearrangement DMAs are ~19x slower).
* On device, substitute z = y / a so the recurrence becomes
      z_t = d * z_{t-1} + x_t,      d = 1 - a
  which is exactly one native `tensor_tensor_scan` (DVE) per
  [128 channels, T] tile — no pre-scale pass over the input.
* The y_0 = x_0 boundary is absorbed into the scan's initial value:
  initial = x_0 / a  gives  z_0 = d*x_0/a + x_0 = x_0/a  =>  y_0 = x_0.
* Post-scale y = a * z runs on the Scalar (ACT) engine with a
  per-partition scale, in parallel with the DVE scans.
"""

import numpy as np

F32 = None  # set lazily (concourse import is heavy; keep module import light)


def _build_program(bpc: int, C: int, T: int, use_broadcast_ap: bool, repeats: int = 1):
    import concourse.bacc as bacc
    import concourse.mybir as mybir
    from concourse.tile import TileContext

    f32 = mybir.dt.float32
    P = 128
    n_ctiles = C // P

    # Bacc (not raw Bass): its finalize() runs generate_event_semaphores,
    # which splits multi-sem waits — TRN2 allows at most 1 wait/instruction.
    nc = bacc.Bacc()
    xt = nc.declare_dram_parameter("xt", [bpc, C, T], f32, isOutput=False)
    a_pd = nc.declare_dram_parameter("a_pd", [P, n_ctiles], f32, isOutput=False)
    d_pd = nc.declare_dram_parameter("d_pd", [P, n_ctiles], f32, isOutput=False)
    ia_pd = nc.declare_dram_parameter("ia_pd", [P, n_ctiles], f32, isOutput=False)
    yt = nc.declare_dram_parameter("yt", [bpc, C, T], f32, isOutput=True)

    with TileContext(nc) as tc:
        with (
            tc.tile_pool(name="coef", bufs=1) as coef_pool,
            tc.tile_pool(name="dbc", bufs=2) as dpool,
            tc.tile_pool(name="xp", bufs=3) as xpool,
            tc.tile_pool(name="zp", bufs=2) as zpool,
            tc.tile_pool(name="yp", bufs=3) as ypool,
            tc.tile_pool(name="init", bufs=4) as spool,
        ):
            a_t = coef_pool.tile([P, n_ctiles], f32, tag="a")
            d_t = coef_pool.tile([P, n_ctiles], f32, tag="d")
            ia_t = coef_pool.tile([P, n_ctiles], f32, tag="ia")
            nc.sync.dma_start(out=a_t[:], in_=a_pd[:, :])
            nc.sync.dma_start(out=d_t[:], in_=d_pd[:, :])
            nc.sync.dma_start(out=ia_t[:], in_=ia_pd[:, :])

            if not use_broadcast_ap:
                ones = coef_pool.tile([P, T], f32, tag="ones")
                nc.vector.memset(ones[:], 1.0)

            for rj in range(repeats * n_ctiles):
                j = rj % n_ctiles
                cs = slice(j * P, (j + 1) * P)
                if use_broadcast_ap:
                    dbc_ap = d_t[:, j : j + 1].broadcast_to([P, T])
                else:
                    dbc = dpool.tile([P, T], f32)
                    # broadcast d[c] along the free dim on the ACT engine
                    nc.scalar.mul(dbc[:], ones[:], d_t[:, j : j + 1])
                    dbc_ap = dbc[:]
                for b in range(bpc):
                    x_tile = xpool.tile([P, T], f32)
                    nc.sync.dma_start(out=x_tile[:], in_=xt[b, cs, :])
                    init = spool.tile([P, 1], f32)
                    nc.vector.tensor_mul(
                        out=init[:], in0=x_tile[:, 0:1], in1=ia_t[:, j : j + 1]
                    )
                    z_tile = zpool.tile([P, T], f32)
                    nc.vector.tensor_tensor_scan(
                        out=z_tile[:],
                        data0=dbc_ap,
                        data1=x_tile[:],
                        initial=init[:],
                        op0=mybir.AluOpType.mult,
                        op1=mybir.AluOpType.add,
                    )
                    y_tile = ypool.tile([P, T], f32)
                    nc.scalar.mul(y_tile[:], z_tile[:], a_t[:, j : j + 1])
                    nc.sync.dma_start(out=yt[b, cs, :], in_=y_tile[:])
    nc.finalize()
    return nc


def _host_coeffs(raw: np.ndarray, P: int = 128):
    """sigmoid / complements in f64, packed [P, n_ctiles] with [p, j] = v[j*P + p]."""
    a64 = 1.0 / (1.0 + np.exp(-raw.astype(np.float64)))
    d64 = 1.0 - a64
    ia64 = 1.0 / a64
    C = raw.shape[0]
    n_ctiles = C // P

    def pack(v64):
        return np.ascontiguousarray(
            v64.astype(np.float32).reshape(n_ctiles, P).T
        )

    return pack(a64), pack(d64), pack(ia64)


USE_BROADCAST_AP = False


def kernel_with_results(
    x: np.ndarray,
    raw: np.ndarray,
    use_broadcast_ap: bool = USE_BROADCAST_AP,
    **run_kwargs,
):
    from concourse.bass_utils import run_bass_kernel_spmd

    B, T, C = x.shape
    n_cores = 8
    bpc = B // n_cores

    a_pd, d_pd, ia_pd = _host_coeffs(raw)

    nc = _build_program(bpc, C, T, use_broadcast_ap=use_broadcast_ap)

    in_maps = []
    for i in range(n_cores):
        shard = np.ascontiguousarray(
            x[i * bpc : (i + 1) * bpc].transpose(0, 2, 1)
        )  # [bpc, C, T], time contiguous
        in_maps.append({"xt": shard, "a_pd": a_pd, "d_pd": d_pd, "ia_pd": ia_pd})

    res = run_bass_kernel_spmd(nc, in_maps, core_ids=list(range(n_cores)), **run_kwargs)

    y = np.empty_like(x)
    for i in range(n_cores):
        y[i * bpc : (i + 1) * bpc] = res.results[i]["yt"].transpose(0, 2, 1)
    return y, res


def kernel(x: np.ndarray, raw: np.ndarray) -> np.ndarray:
    y, _ = kernel_with_results(x, raw)
    return y


# revision 9
# speedup vs baseline: 48.0456x; 17.6318x over previous
"""Trainium2 Bass kernel for channelwise EMA (exponential moving average).

Reference computation (per batch b, channel c):
    a = sigmoid(raw)          # [C]
    y[b, 0, c] = x[b, 0, c]
    y[b, t, c] = a[c] * x[b, t, c] + (1 - a[c]) * y[b, t-1, c]

Strategy
--------
* Shard batch B=16 across 8 cores (2 batches per core); replicate the
  tiny per-channel coefficient vectors.
* Host-side, pre-transpose each core's shard to channel-major
  [bpc, C, T] so time is contiguous — every DMA is then a fully
  contiguous 16KB-per-partition transfer (f32 DMA-transpose does not
  exist on TRN2, and AP-rearrangement DMAs are ~19x slower).
* On device, substitute z = y / a so the recurrence becomes
      z_t = d * z_{t-1} + x_t,      d = 1 - a
  which is exactly one native `tensor_tensor_scan` (DVE) per
  [128 channels, T] tile — no pre-scale pass over the input.
* The y_0 = x_0 boundary is absorbed into the scan's initial value:
  initial = x_0 / a  gives  z_0 = d*x_0/a + x_0 = x_0/a  =>  y_0 = x_0.
* Post-scale y = a * z runs on the Scalar (ACT) engine with a
  per-partition scale, in parallel with the DVE scans.
"""

import numpy as np

F32 = None  # set lazily (concourse import is heavy; keep module import light)


def _build_program(
    bpc: int,
    C: int,
    T: int,
    use_broadcast_ap: bool,
    repeats: int = 1,
    out_dma: str = "sync",
):
    import concourse.bacc as bacc
    import concourse.mybir as mybir
    from concourse.tile import TileContext

    f32 = mybir.dt.float32
    P = 128
    n_ctiles = C // P

    # Bacc (not raw Bass): its finalize() runs generate_event_semaphores,
    # which splits multi-sem waits — TRN2 allows at most 1 wait/instruction.
    nc = bacc.Bacc()
    xt = nc.declare_dram_parameter("xt", [bpc, C, T], f32, isOutput=False)
    a_pd = nc.declare_dram_parameter("a_pd", [P, n_ctiles], f32, isOutput=False)
    d_pd = nc.declare_dram_parameter("d_pd", [P, n_ctiles], f32, isOutput=False)
    ia_pd = nc.declare_dram_parameter("ia_pd", [P, n_ctiles], f32, isOutput=False)
    yt = nc.declare_dram_parameter("yt", [bpc, C, T], f32, isOutput=True)

    with TileContext(nc) as tc:
        with (
            tc.tile_pool(name="coef", bufs=1) as coef_pool,
            tc.tile_pool(name="dbc", bufs=2) as dpool,
            tc.tile_pool(name="xp", bufs=3) as xpool,
            tc.tile_pool(name="zp", bufs=2) as zpool,
            tc.tile_pool(name="yp", bufs=3) as ypool,
            tc.tile_pool(name="init", bufs=4) as spool,
        ):
            a_t = coef_pool.tile([P, n_ctiles], f32, tag="a")
            d_t = coef_pool.tile([P, n_ctiles], f32, tag="d")
            ia_t = coef_pool.tile([P, n_ctiles], f32, tag="ia")
            nc.sync.dma_start(out=a_t[:], in_=a_pd[:, :])
            nc.sync.dma_start(out=d_t[:], in_=d_pd[:, :])
            nc.sync.dma_start(out=ia_t[:], in_=ia_pd[:, :])

            if not use_broadcast_ap:
                ones = coef_pool.tile([P, T], f32, tag="ones")
                nc.vector.memset(ones[:], 1.0)

            for rj in range(repeats * n_ctiles):
                j = rj % n_ctiles
                cs = slice(j * P, (j + 1) * P)
                if use_broadcast_ap:
                    dbc_ap = d_t[:, j : j + 1].broadcast_to([P, T])
                else:
                    dbc = dpool.tile([P, T], f32)
                    # broadcast d[c] along the free dim on the ACT engine
                    nc.scalar.mul(dbc[:], ones[:], d_t[:, j : j + 1])
                    dbc_ap = dbc[:]
                for b in range(bpc):
                    x_tile = xpool.tile([P, T], f32)
                    nc.sync.dma_start(out=x_tile[:], in_=xt[b, cs, :])
                    init = spool.tile([P, 1], f32)
                    nc.vector.tensor_mul(
                        out=init[:], in0=x_tile[:, 0:1], in1=ia_t[:, j : j + 1]
                    )
                    z_tile = zpool.tile([P, T], f32)
                    nc.vector.tensor_tensor_scan(
                        out=z_tile[:],
                        data0=dbc_ap,
                        data1=x_tile[:],
                        initial=init[:],
                        op0=mybir.AluOpType.mult,
                        op1=mybir.AluOpType.add,
                    )
                    y_tile = ypool.tile([P, T], f32)
                    nc.scalar.mul(y_tile[:], z_tile[:], a_t[:, j : j + 1])
                    # out_dma="scalar" puts stores on the second HWDGE ring
                    # (qActDynamicHW) so they don't queue behind loads.
                    getattr(nc, out_dma).dma_start(out=yt[b, cs, :], in_=y_tile[:])
    nc.finalize()
    return nc


def _host_coeffs(raw: np.ndarray, P: int = 128):
    """sigmoid / complements in f64, packed [P, n_ctiles] with [p, j] = v[j*P + p]."""
    a64 = 1.0 / (1.0 + np.exp(-raw.astype(np.float64)))
    d64 = 1.0 - a64
    ia64 = 1.0 / a64
    C = raw.shape[0]
    n_ctiles = C // P

    def pack(v64):
        return np.ascontiguousarray(
            v64.astype(np.float32).reshape(n_ctiles, P).T
        )

    return pack(a64), pack(d64), pack(ia64)


# data0 of the scan as a step-0 broadcast AP (no materialized decay tile):
# HW-validated correct, and measured faster than the materialized variant.
USE_BROADCAST_AP = True


def kernel_with_results(
    x: np.ndarray,
    raw: np.ndarray,
    use_broadcast_ap: bool = USE_BROADCAST_AP,
    **run_kwargs,
):
    from concourse.bass_utils import run_bass_kernel_spmd

    B, T, C = x.shape
    n_cores = 8
    bpc = B // n_cores

    a_pd, d_pd, ia_pd = _host_coeffs(raw)

    nc = _build_program(bpc, C, T, use_broadcast_ap=use_broadcast_ap)

    in_maps = []
    for i in range(n_cores):
        shard = np.ascontiguousarray(
            x[i * bpc : (i + 1) * bpc].transpose(0, 2, 1)
        )  # [bpc, C, T], time contiguous
        in_maps.append({"xt": shard, "a_pd": a_pd, "d_pd": d_pd, "ia_pd": ia_pd})

    res = run_bass_kernel_spmd(nc, in_maps, core_ids=list(range(n_cores)), **run_kwargs)

    y = np.empty_like(x)
    for i in range(n_cores):
        y[i * bpc : (i + 1) * bpc] = res.results[i]["yt"].transpose(0, 2, 1)
    return y, res


def kernel(x: np.ndarray, raw: np.ndarray) -> np.ndarray:
    y, _ = kernel_with_results(x, raw)
    return y
